# revision 29
# baseline (speedup 1.0000x reference)
"""NestedMLP MoE-routed kernel for 8 TRN2 NeuronCores, fp8-accelerated.

Strategy:
  - Host routes tokens by expert (expert_mask), splits each expert's tokens
    across the 8 cores (data-parallel), pads per-core expert groups to a
    common capacity so all cores run one SPMD program.
  - Activations feature-major ([feature, token]) so both matmuls are natural
    lhsT.T @ rhs with contraction on partitions.
  - Precision plan (rel-err gate 2e-2; expert output-norm shares are ~.89/.10/
    .012/.0015 for e3..e0, so the small experts absorb fp8 noise):
      e3: bf16 both layers
      e2: L1 fp8 DoubleRow; L2 fp8 DoubleRow (AGGR) or bf16 (SAFE)
      e1: both layers fp8 DoubleRow
      e0: L1 plain fp8 (K=128 cannot DoubleRow; same speed as bf16 but a
          quarter of the DMA bytes), L2 fp8 DoubleRow
    fp8 weights are pre-scaled by 2^7 on the host (avoids e4m3 subnormals);
    the scale is undone at PSUM eviction (gelu scale=1/128, or the DVE fused
    (ps*1/128)+b2 for the output bias).
  - fp8 DoubleRow matmuls pack two K=128 subtiles per instruction
    (stationary [128,2,128], moving [128,2,cn]) -> 2x bf16 FLOP rate.
  - The DMA front-end ramps slowly (~0.3MB/us for the first ~10us) and the
    rings stay bandwidth-saturated for ~60us; transfers complete roughly in
    ring-entry order. So every tensor is a dedicated dram param transferred
    whole (large contiguous runs) and issued on one queue in exact need
    order, with the first-needed tiles as small as possible. L2 of e0/e1 is
    deferred behind e1-L1 (h8 slots offset) so the 0.5MB w28 is not on the
    critical path of the first matmuls.
  - e0's small remainder chunk runs last so the kernel tail drains a 64-col
    slab instead of a 512-col one.
"""

import math
import os
import sys
import types

sys.path.insert(0, "/opt/trn_rl_repo")

import ml_dtypes
import numpy as np

P = 128
E = 4
D = 1024
H = 4096
OUT = 1024
NCORES = 8
MLP_RATIO = 4

BF16 = ml_dtypes.bfloat16
FP8 = ml_dtypes.float8_e4m3
SW = 128.0  # fp8 weight pre-scale (power of two)

# (d_in, d_hid, d_out) per expert
DIMS = [((D >> (E - 1 - e)), (D >> (E - 1 - e)) * MLP_RATIO, (OUT >> (E - 1 - e))) for e in range(E)]

AGGR = os.environ.get("K_MODE", "aggr") == "aggr"  # e2-L2 in fp8


def _round_up(v, m):
    return ((v + m - 1) // m) * m


def _tile_fmajor(a2d):
    """[F, C] -> [128, F//128, C] with row f = po*128 + pi."""
    f, c = a2d.shape
    return np.ascontiguousarray(a2d.reshape(f // P, P, c).transpose(1, 0, 2))


def _chunks(cap):
    plan, c0 = [], 0
    while c0 < cap:
        cn = min(512, cap - c0)
        plan.append((c0, cn))
        c0 += cn
    return plan


def _build_graph(caps):
    import concourse.mybir as mybir
    import concourse.tile as tile
    from concourse import bacc

    f32 = mybir.dt.float32
    bf16 = mybir.dt.bfloat16
    fp8 = mybir.dt.float8e4
    Gelu = mybir.ActivationFunctionType.Gelu
    DR = mybir.MatmulPerfMode.DoubleRow
    MUL = mybir.AluOpType.mult
    ADD = mybir.AluOpType.add

    ctot = sum(caps)
    offs = np.concatenate([[0], np.cumsum(caps)]).astype(int)

    nc = bacc.Bacc(None, target_bir_lowering=False, debug=False)
    xe0a_d = nc.declare_dram_parameter("xe0a8", [P, 1, 128], fp8, isOutput=False)
    xe0b_d = nc.declare_dram_parameter("xe0b8", [P, 1, caps[0] - 128], fp8, isOutput=False)
    xe1_d = nc.declare_dram_parameter("xe18", [P, 2, caps[1]], fp8, isOutput=False)
    xe2_d = nc.declare_dram_parameter("xe28", [P, 4, caps[2]], fp8, isOutput=False)
    xe3_d = nc.declare_dram_parameter("xe3", [P, 8, caps[3]], bf16, isOutput=False)
    w1e0_d = nc.declare_dram_parameter("w1e08", [P, 1, 512], fp8, isOutput=False)
    w1g_d = nc.declare_dram_parameter("w1g", [P, 8, 8, 512], bf16, isOutput=False)
    w2g_d = nc.declare_dram_parameter("w2g", [P, 4, 32, 256], bf16, isOutput=False)
    w18a_d = nc.declare_dram_parameter("w18a", [P, 2, 1024], fp8, isOutput=False)
    w18b_d = nc.declare_dram_parameter("w18b", [P, 2, 1024], fp8, isOutput=False)
    w18c_d = nc.declare_dram_parameter("w18c", [P, 2, 2048], fp8, isOutput=False)
    w28a_d = nc.declare_dram_parameter("w28a", [P, 8, 256], fp8, isOutput=False)
    w28b1_d = nc.declare_dram_parameter("w28b1", [P, 8, 256], fp8, isOutput=False)
    w28b2_d = nc.declare_dram_parameter("w28b2", [P, 8, 512], fp8, isOutput=False)
    b1_d = nc.declare_dram_parameter("b1t", [P, H // P], f32, isOutput=False)
    b2_d = nc.declare_dram_parameter("b2t", [P, OUT // P], f32, isOutput=False)
    y_d = nc.declare_dram_parameter("yt", [P, OUT // P, ctot], bf16, isOutput=True)

    with tile.TileContext(nc) as tc:
        with (
            tc.tile_pool(name="wpool", bufs=1) as wpool,
            tc.tile_pool(name="xpool", bufs=1) as xpool,
            tc.tile_pool(name="hpool", bufs=1) as hpool,
            tc.tile_pool(name="ypool", bufs=3) as ypool,
            tc.tile_pool(name="pspool", bufs=8, space="PSUM") as pspool,
        ):
            # --- warmup: ramp the PE clock + preload the Gelu table ---
            wu = wpool.tile([P, P], bf16, tag="warmup")
            nc.vector.memset(wu[:], 0.0)
            wact = wpool.tile([P, P], bf16, tag="warmact")
            nc.scalar.activation(wact[:], wu[:], Gelu, bias=0.0)
            for _ in range(8):
                wps = pspool.tile([P, P], f32, tag="ps")
                nc.tensor.matmul(wps[:], wu[:], wu[:], start=True, stop=True)

            b1sb = wpool.tile([P, H // P], f32, tag="b1")
            b2sb = wpool.tile([P, OUT // P], f32, tag="b2")

            w1bx, w2bx, w18x = {}, {}, {}

            def load_whole(eng, xdict, dram, dt, shape, k0, lo, tag):
                """One full-tensor (or full-j-group) DMA -> one SBUF tile,
                registered in xdict at (k0, lo) for wslice lookups."""
                t = wpool.tile([P, *shape], dt, tag=tag, name=tag)
                eng.dma_start(t[:], dram)
                if xdict is not None:
                    for k in range(k0, k0 + shape[0]):
                        xdict.setdefault(k, []).append((lo, lo + shape[1], k0, t))
                return t

            def wslice(xdict, k, mc, width=P):
                for lo, hi, k0, t in xdict[k]:
                    if lo <= mc and mc + width <= hi:
                        return t[:, k - k0, mc - lo : mc - lo + width]
                raise AssertionError("weight slice not found")

            def wpair(xdict, kp, mc, width=P):
                """[128, 2, width] DoubleRow stationary slice."""
                for lo, hi, k0, t in xdict[2 * kp]:
                    if lo <= mc and mc + width <= hi and 2 * kp + 2 - k0 <= t.shape[1]:
                        return t[:, 2 * kp - k0 : 2 * kp - k0 + 2, mc - lo : mc - lo + width]
                raise AssertionError("weight pair slice not found")

            # sync queue, exact need order, first tiles smallest
            xe0b = xpool.tile([P, 1, caps[0] - 128], fp8, tag="xe0b")
            nc.sync.dma_start(xe0b[:], xe0b_d[:])
            xe1 = xpool.tile([P, 2, caps[1]], fp8, tag="xe1")
            nc.sync.dma_start(xe1[:], xe1_d[:])
            load_whole(nc.sync, w18x, w18a_d[:], fp8, [2, 1024], 0, 0, "w18a")
            w28x = {}
            load_whole(nc.sync, w28x, w28a_d[:], fp8, [8, 256], 0, 0, "w28a")
            xe2 = xpool.tile([P, 4, caps[2]], fp8, tag="xe2")
            nc.sync.dma_start(xe2[:], xe2_d[:])
            load_whole(nc.sync, w18x, w18b_d[:], fp8, [2, 1024], 0, 1024, "w18b")
            load_whole(nc.sync, w18x, w18c_d[:], fp8, [2, 2048], 2, 0, "w18c")
            load_whole(nc.sync, w28x, w28b1_d[:], fp8, [8, 256], 0, 256, "w28b1")
            load_whole(nc.sync, w28x, w28b2_d[:], fp8, [8, 512], 8, 0, "w28b2")
            xe3 = xpool.tile([P, 8, caps[3]], bf16, tag="xe3")
            nc.sync.dma_start(xe3[:], xe3_d[:])
            if not AGGR:  # e2's bf16 L2 reads w2 cols 0-512 at ~27us
                for j in range(2):
                    load_whole(nc.sync, w2bx, w2g_d[:, j], bf16, [32, 256], 0, 256 * j, f"w2g{j}")
            for j in range(8):
                load_whole(nc.sync, w1bx, w1g_d[:, j], bf16, [8, 512], 0, 512 * j, f"w1g{j}")
            for j in range(2 if not AGGR else 0, 4):
                load_whole(nc.sync, w2bx, w2g_d[:, j], bf16, [32, 256], 0, 256 * j, f"w2g{j}")

            # scalar queue: tiny early loads, then the engine is all gelu
            xe0a = xpool.tile([P, 1, 128], fp8, tag="xe0a")
            nc.scalar.dma_start(xe0a[:], xe0a_d[:])
            w1e0 = load_whole(nc.scalar, None, w1e0_d[:], fp8, [1, 512], 0, 0, "w1e0")
            nc.scalar.dma_start(b1sb[:], b1_d[:])
            nc.scalar.dma_start(b2sb[:], b2_d[:])

            h8 = hpool.tile([P, 16, 512], fp8, tag="h8")
            hbf = hpool.tile([P, 32, 512], bf16, tag="hbf")

            def w2pair8(kp, mc):
                return wpair(w28x, kp, mc)

            Identity = mybir.ActivationFunctionType.Identity

            def evict_y(ps, m2, col, cn, scaled, on_act=False):
                yt = ypool.tile([P, cn], bf16, tag="yt")
                if on_act:
                    # ACT is idle during e3-L2 (no gelus left) and evicts a
                    # 512-slab in ~0.36us vs DVE's 0.75us — shorter tail
                    nc.scalar.activation(yt[:], ps[:], Identity,
                                         bias=b2sb[:, m2 : m2 + 1],
                                         scale=(1.0 / SW) if scaled else 1.0)
                elif scaled:
                    nc.vector.tensor_scalar(yt[:], ps[:], 1.0 / SW, b2sb[:, m2 : m2 + 1], MUL, ADD)
                else:
                    nc.vector.tensor_scalar_add(yt[:], ps[:], b2sb[:, m2 : m2 + 1])
                nc.sync.dma_start(y_d[:, m2, col : col + cn], yt[:])

            def expert0_l1(xt, tc0, cn, hofs):
                for m in range(4):
                    ps = pspool.tile([P, cn], f32, tag="ps")
                    nc.tensor.matmul(ps[:], w1e0[:, 0, m * P : (m + 1) * P], xt[:, 0, tc0 : tc0 + cn], start=True, stop=True)
                    nc.scalar.activation(h8[:, hofs + m, :cn], ps[:], Gelu, bias=b1sb[:, m : m + 1], scale=1.0 / SW)

            def expert0_l2(c0, cn, hofs):
                col = offs[0] + c0
                ps = pspool.tile([P, cn], f32, tag="ps")
                for kp in range(2):  # K=512
                    nc.tensor.matmul(
                        ps[:], w2pair8(kp, 0), h8[:, hofs + 2 * kp : hofs + 2 * kp + 2, :cn],
                        start=(kp == 0), stop=(kp == 1), perf_mode=DR,
                    )
                evict_y(ps, 0, col, cn, scaled=True)

            # ---- e0 L1 (tiny 128-col chunk first: 16KB of DMA unlocks the
            # first real matmul; the 384-col chunk then fills the wait for
            # e1's tiles) -> e1 L1 (h8 slots 4..11) -> e0 L2s -> e1 L2 ----
            expert0_l1(xe0a, 0, 128, 0)
            cn1 = min(512, caps[0]) - 128
            expert0_l1(xe0b, 0, cn1, 12)

            e1_plan = _chunks(caps[1])
            for c0, cn in e1_plan:
                for m in range(8):
                    ps = pspool.tile([P, cn], f32, tag="ps")
                    nc.tensor.matmul(
                        ps[:], wpair(w18x, 0, m * P), xe1[:, 0:2, c0 : c0 + cn],
                        start=True, stop=True, perf_mode=DR,
                    )
                    nc.scalar.activation(h8[:, 4 + m, :cn], ps[:], Gelu, bias=b1sb[:, m : m + 1], scale=1.0 / SW)

            expert0_l2(0, 128, 0)
            expert0_l2(128, cn1, 12)

            for c0, cn in e1_plan:
                col = offs[1] + c0
                for m2 in range(2):
                    ps = pspool.tile([P, cn], f32, tag="ps")
                    for kp in range(4):  # K=1024
                        nc.tensor.matmul(
                            ps[:], w2pair8(kp, m2 * P), h8[:, 4 + 2 * kp : 4 + 2 * kp + 2, :cn],
                            start=(kp == 0), stop=(kp == 3), perf_mode=DR,
                        )
                    evict_y(ps, m2, col, cn, scaled=True)

            # ---- expert 2: L1 fp8 DR; L2 fp8 DR (AGGR) or bf16 ----
            for c0, cn in _chunks(caps[2]):
                col = offs[2] + c0
                for m in range(16):
                    ps = pspool.tile([P, cn], f32, tag="ps")
                    for kp in range(2):  # K=512
                        nc.tensor.matmul(
                            ps[:], wpair(w18x, kp, m * P), xe2[:, 2 * kp : 2 * kp + 2, c0 : c0 + cn],
                            start=(kp == 0), stop=(kp == 1), perf_mode=DR,
                        )
                    if AGGR:
                        nc.scalar.activation(h8[:, m, :cn], ps[:], Gelu, bias=b1sb[:, m : m + 1], scale=1.0 / SW)
                    else:
                        nc.scalar.activation(hbf[:, m, :cn], ps[:], Gelu, bias=b1sb[:, m : m + 1], scale=1.0 / SW)
                for m2 in range(4):
                    ps = pspool.tile([P, cn], f32, tag="ps")
                    if AGGR:
                        for kp in range(8):  # K=2048
                            nc.tensor.matmul(
                                ps[:], w2pair8(kp, m2 * P), h8[:, 2 * kp : 2 * kp + 2, :cn],
                                start=(kp == 0), stop=(kp == 7), perf_mode=DR,
                            )
                        evict_y(ps, m2, col, cn, scaled=True)
                    else:
                        for k in range(16):
                            nc.tensor.matmul(
                                ps[:], wslice(w2bx, k, m2 * P), hbf[:, k, :cn],
                                start=(k == 0), stop=(k == 15),
                            )
                        evict_y(ps, m2, col, cn, scaled=False)

            # ---- expert 3: bf16 both layers ----
            for c0, cn in _chunks(caps[3]):
                col = offs[3] + c0
                for m in range(32):
                    ps = pspool.tile([P, cn], f32, tag="ps")
                    for k in range(8):
                        nc.tensor.matmul(
                            ps[:], wslice(w1bx, k, m * P), xe3[:, k, c0 : c0 + cn],
                            start=(k == 0), stop=(k == 7),
                        )
                    nc.scalar.activation(hbf[:, m, :cn], ps[:], Gelu, bias=b1sb[:, m : m + 1])
                for m2 in range(8):
                    ps = pspool.tile([P, cn], f32, tag="ps")
                    for k in range(32):
                        nc.tensor.matmul(
                            ps[:], wslice(w2bx, k, m2 * P), hbf[:, k, :cn],
                            start=(k == 0), stop=(k == 31),
                        )
                    evict_y(ps, m2, col, cn, scaled=False)

            # ---- expert 0 remainder: tiny tail drain ----
            for c0, cn in _chunks(caps[0])[1:]:
                expert0_l1(xe0b, c0 - 128, cn, 0)
                expert0_l2(c0, cn, 0)

    nc.compile()
    return nc, ctot, offs


def _ensure_ntff_hook_importable():
    try:
        import antenv.axon_hooks  # noqa: F401
        return
    except ImportError:
        pass
    holder = {"hook": None}
    m = types.ModuleType("antenv.axon_hooks")
    m.set_axon_ntff_profile_hook = lambda h: holder.__setitem__("hook", h)
    m.get_axon_ntff_profile_hook = lambda: holder["hook"]
    sys.modules["antenv.axon_hooks"] = m
    try:
        from trn_agent_boot.trn_boot import _ntff_profile_via_ctypes

        m.set_axon_ntff_profile_hook(_ntff_profile_via_ctypes("/opt/axon/libaxon_pjrt.so"))
    except Exception:
        pass


def kernel(x, expert_mask, w1, b1, w2, b2):
    _ensure_ntff_hook_importable()
    from concourse.bass_utils import run_bass_kernel_spmd

    B, N, _ = x.shape
    T = B * N
    xf = np.asarray(x, dtype=np.float32).reshape(T, D)
    mask = np.asarray(expert_mask).reshape(T).astype(np.int64)

    # --- host routing ---
    ids_by_e = [np.nonzero(mask == e)[0] for e in range(E)]
    counts = [len(i) for i in ids_by_e]
    caps = [max(64, _round_up(math.ceil(c / NCORES), 64)) for c in counts]
    core_ids = [[None] * E for _ in range(NCORES)]
    for e in range(E):
        parts = np.array_split(ids_by_e[e], NCORES)
        for c in range(NCORES):
            assert len(parts[c]) <= caps[e]
            core_ids[c][e] = parts[c]

    nc, ctot, offs = _build_graph(caps)

    # --- host weight prep ---
    w1f = np.asarray(w1, np.float32)
    w2f = np.asarray(w2, np.float32)
    w1bt = _tile_fmajor(w1f.T).astype(BF16)                             # [128, 8, 4096]
    w2bt = _tile_fmajor(w2f.T).astype(BF16)                             # [128, 32, 1024]
    w1g = np.ascontiguousarray(w1bt.reshape(P, 8, 8, 512).transpose(0, 2, 1, 3))
    w2g = np.ascontiguousarray(w2bt.reshape(P, 32, 4, 256).transpose(0, 2, 1, 3))
    w18t = _tile_fmajor((w1f[:2048, :512] * SW).T).astype(FP8)          # [128, 4, 2048]
    w1e0 = np.ascontiguousarray(w18t[:, 0:1, 0:512])
    w18a = np.ascontiguousarray(w18t[:, 0:2, 0:1024])
    w18b = np.ascontiguousarray(w18t[:, 0:2, 1024:2048])
    w18c = np.ascontiguousarray(w18t[:, 2:4, :])
    w28t = _tile_fmajor((w2f[:512, :2048] * SW).T).astype(FP8)          # [128, 16, 512]
    w28a = np.ascontiguousarray(w28t[:, 0:8, 0:256])
    w28b1 = np.ascontiguousarray(w28t[:, 0:8, 256:512])
    w28b2 = np.ascontiguousarray(w28t[:, 8:16, :])
    b1t = np.ascontiguousarray(np.asarray(b1, np.float32).reshape(H // P, P).T)
    b2t = np.ascontiguousarray(np.asarray(b2, np.float32).reshape(OUT // P, P).T)

    in_maps = []
    for c in range(NCORES):
        ids = core_ids[c]
        xg0 = np.zeros((caps[0], P), np.float32)
        xg0[: len(ids[0])] = xf[ids[0]][:, :P]
        xe0 = _tile_fmajor(xg0.T).astype(FP8)                           # [128, 1, cap0]
        xe0a = np.ascontiguousarray(xe0[:, :, :128])
        xe0b = np.ascontiguousarray(xe0[:, :, 128:])
        xg1 = np.zeros((caps[1], 256), np.float32)
        xg1[: len(ids[1])] = xf[ids[1]][:, :256]
        xe1 = _tile_fmajor(xg1.T).astype(FP8)                           # [128, 2, cap1]
        xg2 = np.zeros((caps[2], 512), np.float32)
        xg2[: len(ids[2])] = xf[ids[2]][:, :512]
        xe2 = _tile_fmajor(xg2.T).astype(FP8)                           # [128, 4, cap2]
        xg3 = np.zeros((caps[3], D), np.float32)
        xg3[: len(ids[3])] = xf[ids[3]]
        xe3 = _tile_fmajor(xg3.T).astype(BF16)                          # [128, 8, cap3]

        in_maps.append(
            {"xe0a8": xe0a, "xe0b8": xe0b, "xe18": xe1, "xe28": xe2, "xe3": xe3,
             "w1e08": w1e0, "w1g": w1g, "w2g": w2g, "w18a": w18a,
             "w18b": w18b, "w18c": w18c, "w28a": w28a, "w28b1": w28b1,
             "w28b2": w28b2, "b1t": b1t, "b2t": b2t}
        )

    res = run_bass_kernel_spmd(nc, in_maps, list(range(NCORES)))

    # --- host output assembly ---
    y = np.zeros((T, OUT), np.float32)
    for c in range(NCORES):
        yr = np.asarray(res.results[c]["yt"]).astype(np.float32)        # [128, 8, ctot]
        yfull = yr.transpose(1, 0, 2).reshape(OUT, ctot)
        for e in range(E):
            d_out = DIMS[e][2]
            ids = core_ids[c][e]
            if len(ids):
                y[ids, :d_out] = yfull[:d_out, offs[e] : offs[e] + len(ids)].T
    return y.reshape(B, N, OUT)


# revision 30
# speedup vs baseline: 1.0127x; 1.0127x over previous
"""NestedMLP MoE-routed kernel for 8 TRN2 NeuronCores, fp8-accelerated.

Strategy:
  - Host routes tokens by expert (expert_mask), splits each expert's tokens
    across the 8 cores (data-parallel), pads per-core expert groups to a
    common capacity so all cores run one SPMD program.
  - Activations feature-major ([feature, token]) so both matmuls are natural
    lhsT.T @ rhs with contraction on partitions.
  - Precision plan (rel-err gate 2e-2; expert output-norm shares are ~.89/.10/
    .012/.0015 for e3..e0, so the small experts absorb fp8 noise):
      e3: bf16 both layers
      e2: L1 fp8 DoubleRow; L2 fp8 DoubleRow (AGGR) or bf16 (SAFE)
      e1: both layers fp8 DoubleRow
      e0: L1 plain fp8 (K=128 cannot DoubleRow; same speed as bf16 but a
          quarter of the DMA bytes), L2 fp8 DoubleRow
    fp8 weights are pre-scaled by 2^7 on the host (avoids e4m3 subnormals);
    the scale is undone at PSUM eviction (gelu scale=1/128, or the DVE fused
    (ps*1/128)+b2 for the output bias).
  - fp8 DoubleRow matmuls pack two K=128 subtiles per instruction
    (stationary [128,2,128], moving [128,2,cn]) -> 2x bf16 FLOP rate.
  - The DMA front-end ramps slowly (~0.3MB/us for the first ~10us) and the
    rings stay bandwidth-saturated for ~60us; transfers complete roughly in
    ring-entry order. So every tensor is a dedicated dram param transferred
    whole (large contiguous runs) and issued on one queue in exact need
    order, with the first-needed tiles as small as possible. L2 of e0/e1 is
    deferred behind e1-L1 (h8 slots offset) so the 0.5MB w28 is not on the
    critical path of the first matmuls.
  - e0's small remainder chunk runs last so the kernel tail drains a 64-col
    slab instead of a 512-col one.
"""

import math
import os
import sys
import types

sys.path.insert(0, "/opt/trn_rl_repo")

import ml_dtypes
import numpy as np

P = 128
E = 4
D = 1024
H = 4096
OUT = 1024
NCORES = 8
MLP_RATIO = 4

BF16 = ml_dtypes.bfloat16
FP8 = ml_dtypes.float8_e4m3
SW = 128.0  # fp8 weight pre-scale (power of two)

# (d_in, d_hid, d_out) per expert
DIMS = [((D >> (E - 1 - e)), (D >> (E - 1 - e)) * MLP_RATIO, (OUT >> (E - 1 - e))) for e in range(E)]

AGGR = os.environ.get("K_MODE", "aggr") == "aggr"  # e2-L2 in fp8


def _round_up(v, m):
    return ((v + m - 1) // m) * m


def _tile_fmajor(a2d):
    """[F, C] -> [128, F//128, C] with row f = po*128 + pi."""
    f, c = a2d.shape
    return np.ascontiguousarray(a2d.reshape(f // P, P, c).transpose(1, 0, 2))


def _chunks(cap):
    plan, c0 = [], 0
    while c0 < cap:
        cn = min(512, cap - c0)
        plan.append((c0, cn))
        c0 += cn
    return plan


def _build_graph(caps):
    import concourse.mybir as mybir
    import concourse.tile as tile
    from concourse import bacc

    f32 = mybir.dt.float32
    bf16 = mybir.dt.bfloat16
    fp8 = mybir.dt.float8e4
    Gelu = mybir.ActivationFunctionType.Gelu
    DR = mybir.MatmulPerfMode.DoubleRow
    MUL = mybir.AluOpType.mult
    ADD = mybir.AluOpType.add

    ctot = sum(caps)
    offs = np.concatenate([[0], np.cumsum(caps)]).astype(int)

    nc = bacc.Bacc(None, target_bir_lowering=False, debug=False)
    xe0_d = nc.declare_dram_parameter("xe08", [P, 1, caps[0]], fp8, isOutput=False)
    xe1_d = nc.declare_dram_parameter("xe18", [P, 2, caps[1]], fp8, isOutput=False)
    xe2_d = nc.declare_dram_parameter("xe28", [P, 4, caps[2]], fp8, isOutput=False)
    xe3_d = nc.declare_dram_parameter("xe3", [P, 8, caps[3]], bf16, isOutput=False)
    w1e0_d = nc.declare_dram_parameter("w1e08", [P, 1, 512], fp8, isOutput=False)
    w1g_d = nc.declare_dram_parameter("w1g", [P, 8, 8, 512], bf16, isOutput=False)
    w2g_d = nc.declare_dram_parameter("w2g", [P, 4, 32, 256], bf16, isOutput=False)
    w18a_d = nc.declare_dram_parameter("w18a", [P, 2, 1024], fp8, isOutput=False)
    w18b_d = nc.declare_dram_parameter("w18b", [P, 2, 1024], fp8, isOutput=False)
    w18c_d = nc.declare_dram_parameter("w18c", [P, 2, 2048], fp8, isOutput=False)
    w28a_d = nc.declare_dram_parameter("w28a", [P, 8, 256], fp8, isOutput=False)
    w28b1_d = nc.declare_dram_parameter("w28b1", [P, 8, 256], fp8, isOutput=False)
    w28b2_d = nc.declare_dram_parameter("w28b2", [P, 8, 512], fp8, isOutput=False)
    b1_d = nc.declare_dram_parameter("b1t", [P, H // P], f32, isOutput=False)
    b2_d = nc.declare_dram_parameter("b2t", [P, OUT // P], f32, isOutput=False)
    y_d = nc.declare_dram_parameter("yt", [P, OUT // P, ctot], bf16, isOutput=True)

    with tile.TileContext(nc) as tc:
        with (
            tc.tile_pool(name="wpool", bufs=1) as wpool,
            tc.tile_pool(name="xpool", bufs=1) as xpool,
            tc.tile_pool(name="hpool", bufs=1) as hpool,
            tc.tile_pool(name="ypool", bufs=3) as ypool,
            tc.tile_pool(name="pspool", bufs=8, space="PSUM") as pspool,
        ):
            # --- warmup: ramp the PE clock + preload the Gelu table ---
            wu = wpool.tile([P, P], bf16, tag="warmup")
            nc.vector.memset(wu[:], 0.0)
            wact = wpool.tile([P, P], bf16, tag="warmact")
            nc.scalar.activation(wact[:], wu[:], Gelu, bias=0.0)
            for _ in range(8):
                wps = pspool.tile([P, P], f32, tag="ps")
                nc.tensor.matmul(wps[:], wu[:], wu[:], start=True, stop=True)

            b1sb = wpool.tile([P, H // P], f32, tag="b1")
            b2sb = wpool.tile([P, OUT // P], f32, tag="b2")

            w1bx, w2bx, w18x = {}, {}, {}

            def load_whole(eng, xdict, dram, dt, shape, k0, lo, tag):
                """One full-tensor (or full-j-group) DMA -> one SBUF tile,
                registered in xdict at (k0, lo) for wslice lookups."""
                t = wpool.tile([P, *shape], dt, tag=tag, name=tag)
                eng.dma_start(t[:], dram)
                if xdict is not None:
                    for k in range(k0, k0 + shape[0]):
                        xdict.setdefault(k, []).append((lo, lo + shape[1], k0, t))
                return t

            def wslice(xdict, k, mc, width=P):
                for lo, hi, k0, t in xdict[k]:
                    if lo <= mc and mc + width <= hi:
                        return t[:, k - k0, mc - lo : mc - lo + width]
                raise AssertionError("weight slice not found")

            def wpair(xdict, kp, mc, width=P):
                """[128, 2, width] DoubleRow stationary slice."""
                for lo, hi, k0, t in xdict[2 * kp]:
                    if lo <= mc and mc + width <= hi and 2 * kp + 2 - k0 <= t.shape[1]:
                        return t[:, 2 * kp - k0 : 2 * kp - k0 + 2, mc - lo : mc - lo + width]
                raise AssertionError("weight pair slice not found")

            # sync queue, exact need order, first tiles smallest
            xe0 = xpool.tile([P, 1, caps[0]], fp8, tag="xe0")
            nc.sync.dma_start(xe0[:], xe0_d[:])
            xe1 = xpool.tile([P, 2, caps[1]], fp8, tag="xe1")
            nc.sync.dma_start(xe1[:], xe1_d[:])
            load_whole(nc.sync, w18x, w18a_d[:], fp8, [2, 1024], 0, 0, "w18a")
            w28x = {}
            load_whole(nc.sync, w28x, w28a_d[:], fp8, [8, 256], 0, 0, "w28a")
            xe2 = xpool.tile([P, 4, caps[2]], fp8, tag="xe2")
            nc.sync.dma_start(xe2[:], xe2_d[:])
            load_whole(nc.sync, w18x, w18b_d[:], fp8, [2, 1024], 0, 1024, "w18b")
            load_whole(nc.sync, w18x, w18c_d[:], fp8, [2, 2048], 2, 0, "w18c")
            load_whole(nc.sync, w28x, w28b1_d[:], fp8, [8, 256], 0, 256, "w28b1")
            load_whole(nc.sync, w28x, w28b2_d[:], fp8, [8, 512], 8, 0, "w28b2")
            xe3 = xpool.tile([P, 8, caps[3]], bf16, tag="xe3")
            nc.sync.dma_start(xe3[:], xe3_d[:])
            if not AGGR:  # e2's bf16 L2 reads w2 cols 0-512 at ~27us
                for j in range(2):
                    load_whole(nc.sync, w2bx, w2g_d[:, j], bf16, [32, 256], 0, 256 * j, f"w2g{j}")
            for j in range(8):
                load_whole(nc.sync, w1bx, w1g_d[:, j], bf16, [8, 512], 0, 512 * j, f"w1g{j}")
            for j in range(2 if not AGGR else 0, 4):
                load_whole(nc.sync, w2bx, w2g_d[:, j], bf16, [32, 256], 0, 256 * j, f"w2g{j}")

            # scalar queue: tiny early loads, then the engine is all gelu
            w1e0 = load_whole(nc.scalar, None, w1e0_d[:], fp8, [1, 512], 0, 0, "w1e0")
            nc.scalar.dma_start(b1sb[:], b1_d[:])
            nc.scalar.dma_start(b2sb[:], b2_d[:])

            h8 = hpool.tile([P, 16, 512], fp8, tag="h8")
            hbf = hpool.tile([P, 32, 512], bf16, tag="hbf")

            def w2pair8(kp, mc):
                return wpair(w28x, kp, mc)

            Identity = mybir.ActivationFunctionType.Identity

            def evict_y(ps, m2, col, cn, scaled, on_act=False):
                yt = ypool.tile([P, cn], bf16, tag="yt")
                if on_act:
                    # ACT is idle during e3-L2 (no gelus left) and evicts a
                    # 512-slab in ~0.36us vs DVE's 0.75us — shorter tail
                    nc.scalar.activation(yt[:], ps[:], Identity,
                                         bias=b2sb[:, m2 : m2 + 1],
                                         scale=(1.0 / SW) if scaled else 1.0)
                elif scaled:
                    nc.vector.tensor_scalar(yt[:], ps[:], 1.0 / SW, b2sb[:, m2 : m2 + 1], MUL, ADD)
                else:
                    nc.vector.tensor_scalar_add(yt[:], ps[:], b2sb[:, m2 : m2 + 1])
                nc.sync.dma_start(y_d[:, m2, col : col + cn], yt[:])

            def expert0_l1(xt, tc0, cn, hofs):
                for m in range(4):
                    ps = pspool.tile([P, cn], f32, tag="ps")
                    nc.tensor.matmul(ps[:], w1e0[:, 0, m * P : (m + 1) * P], xt[:, 0, tc0 : tc0 + cn], start=True, stop=True)
                    nc.scalar.activation(h8[:, hofs + m, :cn], ps[:], Gelu, bias=b1sb[:, m : m + 1], scale=1.0 / SW)

            def expert0_l2(c0, cn, hofs):
                col = offs[0] + c0
                ps = pspool.tile([P, cn], f32, tag="ps")
                for kp in range(2):  # K=512
                    nc.tensor.matmul(
                        ps[:], w2pair8(kp, 0), h8[:, hofs + 2 * kp : hofs + 2 * kp + 2, :cn],
                        start=(kp == 0), stop=(kp == 1), perf_mode=DR,
                    )
                evict_y(ps, 0, col, cn, scaled=True)

            # ---- e0 chunk0 L1 -> e1 L1 (h8 slots 4..11) -> e0 L2 -> e1 L2
            # (defers the w28-dependent work behind the smallest DMA prefix)
            e0_plan = _chunks(caps[0])
            c00, cn0 = e0_plan[0]
            expert0_l1(xe0, c00, cn0, 0)

            e1_plan = _chunks(caps[1])
            for c0, cn in e1_plan:
                for m in range(8):
                    ps = pspool.tile([P, cn], f32, tag="ps")
                    nc.tensor.matmul(
                        ps[:], wpair(w18x, 0, m * P), xe1[:, 0:2, c0 : c0 + cn],
                        start=True, stop=True, perf_mode=DR,
                    )
                    nc.scalar.activation(h8[:, 4 + m, :cn], ps[:], Gelu, bias=b1sb[:, m : m + 1], scale=1.0 / SW)

            expert0_l2(c00, cn0, 0)

            for c0, cn in e1_plan:
                col = offs[1] + c0
                for m2 in range(2):
                    ps = pspool.tile([P, cn], f32, tag="ps")
                    for kp in range(4):  # K=1024
                        nc.tensor.matmul(
                            ps[:], w2pair8(kp, m2 * P), h8[:, 4 + 2 * kp : 4 + 2 * kp + 2, :cn],
                            start=(kp == 0), stop=(kp == 3), perf_mode=DR,
                        )
                    evict_y(ps, m2, col, cn, scaled=True)

            # ---- expert 2: L1 fp8 DR; L2 fp8 DR (AGGR) or bf16 ----
            for c0, cn in _chunks(caps[2]):
                col = offs[2] + c0
                for m in range(16):
                    ps = pspool.tile([P, cn], f32, tag="ps")
                    for kp in range(2):  # K=512
                        nc.tensor.matmul(
                            ps[:], wpair(w18x, kp, m * P), xe2[:, 2 * kp : 2 * kp + 2, c0 : c0 + cn],
                            start=(kp == 0), stop=(kp == 1), perf_mode=DR,
                        )
                    if AGGR:
                        nc.scalar.activation(h8[:, m, :cn], ps[:], Gelu, bias=b1sb[:, m : m + 1], scale=1.0 / SW)
                    else:
                        nc.scalar.activation(hbf[:, m, :cn], ps[:], Gelu, bias=b1sb[:, m : m + 1], scale=1.0 / SW)
                for m2 in range(4):
                    ps = pspool.tile([P, cn], f32, tag="ps")
                    if AGGR:
                        for kp in range(8):  # K=2048
                            nc.tensor.matmul(
                                ps[:], w2pair8(kp, m2 * P), h8[:, 2 * kp : 2 * kp + 2, :cn],
                                start=(kp == 0), stop=(kp == 7), perf_mode=DR,
                            )
                        evict_y(ps, m2, col, cn, scaled=True)
                    else:
                        for k in range(16):
                            nc.tensor.matmul(
                                ps[:], wslice(w2bx, k, m2 * P), hbf[:, k, :cn],
                                start=(k == 0), stop=(k == 15),
                            )
                        evict_y(ps, m2, col, cn, scaled=False)

            # ---- expert 3: bf16 both layers ----
            for c0, cn in _chunks(caps[3]):
                col = offs[3] + c0
                for m in range(32):
                    ps = pspool.tile([P, cn], f32, tag="ps")
                    for k in range(8):
                        nc.tensor.matmul(
                            ps[:], wslice(w1bx, k, m * P), xe3[:, k, c0 : c0 + cn],
                            start=(k == 0), stop=(k == 7),
                        )
                    nc.scalar.activation(hbf[:, m, :cn], ps[:], Gelu, bias=b1sb[:, m : m + 1])
                for m2 in range(8):
                    ps = pspool.tile([P, cn], f32, tag="ps")
                    for k in range(32):
                        nc.tensor.matmul(
                            ps[:], wslice(w2bx, k, m2 * P), hbf[:, k, :cn],
                            start=(k == 0), stop=(k == 31),
                        )
                    evict_y(ps, m2, col, cn, scaled=False)

            # ---- expert 0 remainder: tiny tail drain ----
            for c0, cn in e0_plan[1:]:
                expert0_l1(xe0, c0, cn, 0)
                expert0_l2(c0, cn, 0)

    nc.compile()
    return nc, ctot, offs


def _ensure_ntff_hook_importable():
    try:
        import antenv.axon_hooks  # noqa: F401
        return
    except ImportError:
        pass
    holder = {"hook": None}
    m = types.ModuleType("antenv.axon_hooks")
    m.set_axon_ntff_profile_hook = lambda h: holder.__setitem__("hook", h)
    m.get_axon_ntff_profile_hook = lambda: holder["hook"]
    sys.modules["antenv.axon_hooks"] = m
    try:
        from trn_agent_boot.trn_boot import _ntff_profile_via_ctypes

        m.set_axon_ntff_profile_hook(_ntff_profile_via_ctypes("/opt/axon/libaxon_pjrt.so"))
    except Exception:
        pass


def kernel(x, expert_mask, w1, b1, w2, b2):
    _ensure_ntff_hook_importable()
    from concourse.bass_utils import run_bass_kernel_spmd

    B, N, _ = x.shape
    T = B * N
    xf = np.asarray(x, dtype=np.float32).reshape(T, D)
    mask = np.asarray(expert_mask).reshape(T).astype(np.int64)

    # --- host routing ---
    ids_by_e = [np.nonzero(mask == e)[0] for e in range(E)]
    counts = [len(i) for i in ids_by_e]
    caps = [max(64, _round_up(math.ceil(c / NCORES), 64)) for c in counts]
    core_ids = [[None] * E for _ in range(NCORES)]
    for e in range(E):
        parts = np.array_split(ids_by_e[e], NCORES)
        for c in range(NCORES):
            assert len(parts[c]) <= caps[e]
            core_ids[c][e] = parts[c]

    nc, ctot, offs = _build_graph(caps)

    # --- host weight prep ---
    w1f = np.asarray(w1, np.float32)
    w2f = np.asarray(w2, np.float32)
    w1bt = _tile_fmajor(w1f.T).astype(BF16)                             # [128, 8, 4096]
    w2bt = _tile_fmajor(w2f.T).astype(BF16)                             # [128, 32, 1024]
    w1g = np.ascontiguousarray(w1bt.reshape(P, 8, 8, 512).transpose(0, 2, 1, 3))
    w2g = np.ascontiguousarray(w2bt.reshape(P, 32, 4, 256).transpose(0, 2, 1, 3))
    w18t = _tile_fmajor((w1f[:2048, :512] * SW).T).astype(FP8)          # [128, 4, 2048]
    w1e0 = np.ascontiguousarray(w18t[:, 0:1, 0:512])
    w18a = np.ascontiguousarray(w18t[:, 0:2, 0:1024])
    w18b = np.ascontiguousarray(w18t[:, 0:2, 1024:2048])
    w18c = np.ascontiguousarray(w18t[:, 2:4, :])
    w28t = _tile_fmajor((w2f[:512, :2048] * SW).T).astype(FP8)          # [128, 16, 512]
    w28a = np.ascontiguousarray(w28t[:, 0:8, 0:256])
    w28b1 = np.ascontiguousarray(w28t[:, 0:8, 256:512])
    w28b2 = np.ascontiguousarray(w28t[:, 8:16, :])
    b1t = np.ascontiguousarray(np.asarray(b1, np.float32).reshape(H // P, P).T)
    b2t = np.ascontiguousarray(np.asarray(b2, np.float32).reshape(OUT // P, P).T)

    in_maps = []
    for c in range(NCORES):
        ids = core_ids[c]
        xg0 = np.zeros((caps[0], P), np.float32)
        xg0[: len(ids[0])] = xf[ids[0]][:, :P]
        xe0 = _tile_fmajor(xg0.T).astype(FP8)                           # [128, 1, cap0]
        xg1 = np.zeros((caps[1], 256), np.float32)
        xg1[: len(ids[1])] = xf[ids[1]][:, :256]
        xe1 = _tile_fmajor(xg1.T).astype(FP8)                           # [128, 2, cap1]
        xg2 = np.zeros((caps[2], 512), np.float32)
        xg2[: len(ids[2])] = xf[ids[2]][:, :512]
        xe2 = _tile_fmajor(xg2.T).astype(FP8)                           # [128, 4, cap2]
        xg3 = np.zeros((caps[3], D), np.float32)
        xg3[: len(ids[3])] = xf[ids[3]]
        xe3 = _tile_fmajor(xg3.T).astype(BF16)                          # [128, 8, cap3]

        in_maps.append(
            {"xe08": xe0, "xe18": xe1, "xe28": xe2, "xe3": xe3,
             "w1e08": w1e0, "w1g": w1g, "w2g": w2g, "w18a": w18a,
             "w18b": w18b, "w18c": w18c, "w28a": w28a, "w28b1": w28b1,
             "w28b2": w28b2, "b1t": b1t, "b2t": b2t}
        )

    res = run_bass_kernel_spmd(nc, in_maps, list(range(NCORES)))

    # --- host output assembly ---
    y = np.zeros((T, OUT), np.float32)
    for c in range(NCORES):
        yr = np.asarray(res.results[c]["yt"]).astype(np.float32)        # [128, 8, ctot]
        yfull = yr.transpose(1, 0, 2).reshape(OUT, ctot)
        for e in range(E):
            d_out = DIMS[e][2]
            ids = core_ids[c][e]
            if len(ids):
                y[ids, :d_out] = yfull[:d_out, offs[e] : offs[e] + len(ids)].T
    return y.reshape(B, N, OUT)
